# revision 13
# baseline (speedup 1.0000x reference)
"""Trainium2 Bass kernel for the ConvS2S-style decoder (nn_Decoder).

Strategy: pure data-parallel over batch — B=8 batch elements mapped 1:1 onto
8 NeuronCores, zero cross-core communication.  Host does the (tiny) embedding
gather + weight transposes/relayouts; each core runs the full per-batch
pipeline:

    u0 = emb @ W_e2h computed on HOST (numpy); u0 (f32) + its bf16 copy
    stream to the device, so layer-0 conv starts as soon as 1MB lands
    6 x [ conv(K=3, via 3 shifted matmuls) -> GLU
          -> attention (energy in (S,T) layout, softmax column sums via a
             2.0-constant ones matmul so the reciprocal runs full-partition,
             normalization folded into the att2 epilogue) -> residual updates ]
    convout = u.T @ W_h2e ; out = convout @ W_fc   (T,V layout, bf16 out)

All matmuls are bf16 x bf16.  Key perf detail: conv/fc weights stream from HBM in
mega-tiles with 8-16KB contiguous per-partition rows — small (2KB) DMA
descriptors at the conv's ~120GB/s stream rate slow PE SBUF reads by ~20%
(259ns vs 216ns per 512-col matmul).  The residual u stays f32; everything
else is bf16, halving HBM traffic vs f32.
"""

import numpy as np
from contextlib import ExitStack

import bass_rust
import concourse.bass as bass
import concourse.mybir as mybir
import concourse.tile as tile
from concourse.alu_op_type import AluOpType

F32 = mybir.dt.float32
F32R = mybir.dt.float32r
BF16 = mybir.dt.bfloat16
AF = mybir.ActivationFunctionType
P = 128

_last_results = None


def _legalize_pe_waits(nc):
    """Walrus packs a self-loading (fp32/fp32r) Matmult's sync waits into the
    LDWEIGHTS hw descriptor, which has a single wait slot.  Move the waits of
    any multi-wait PE compute instruction onto EventSemaphore instructions
    (one wait each) inserted just before it on the PE queue — semantically
    identical wait point, but each carrier is within the hw limit."""
    n = 0
    absorb_types = (
        "InstMatmult",
        "InstLdweights",
        "InstDMACopy",
        "InstActivation",
        "InstTensorTensor",
        "InstTensorScalarPtr",
        "InstTensorCopy",
        "InstReciprocal",
        "InstMemset",
        "InstTensorReduce",
        "InstDrain",
    )
    for fn in nc.m.functions:
        for blk in fn.blocks:
            out = []
            changed = False
            for inst in blk.instructions:
                si = inst.sync_info
                if (
                    si is not None
                    and type(inst).__name__ in absorb_types
                ):
                    waits = list(si.on_wait)
                    if len(waits) > 1:
                        for w in waits:
                            out.append(
                                mybir.InstEventSemaphore(
                                    name=f"I-pewait{n}",
                                    engine=inst.engine,
                                    sync_info=bass_rust.SyncInfo(
                                        on_wait=[w], on_update=[]
                                    ),
                                    ins=[],
                                    outs=[],
                                )
                            )
                            n += 1
                        inst.sync_info = bass_rust.SyncInfo(
                            on_wait=[], on_update=list(si.on_update)
                        )
                        changed = True
                out.append(inst)
            if changed:
                blk.instructions = out
    return n


def build_decoder_nc(T, S, E, H, V, L, KW, CH, with_bias, legalize=True):
    """Build the per-core Bass program.  All dims must be multiples of 128
    (except V which must be a multiple of CH, CH <= 512)."""
    kE, kH, kS, mT = E // P, H // P, S // P, T // P
    NCH = V // CH
    KG = 4  # k-tiles batched per conv weight mega-tile DMA
    GS = 4 if NCH % 4 == 0 else (2 if NCH % 2 == 0 else 1)
    GW = GS * CH
    NCG = NCH // GS
    SQ = float(np.sqrt(np.float32(0.5)))
    S2 = 0.5  # SQ**2 exactly
    RT2 = float(np.float32(np.sqrt(np.float32(2.0))))

    nc = bass.Bass()

    UW = T + KW - 1
    d_u0 = nc.declare_dram_parameter("u0", [P, kH * UW], F32R, isOutput=False)
    d_ubf0 = nc.declare_dram_parameter("ubf0", [P, kH * UW], BF16, isOutput=False)
    d_embs = nc.declare_dram_parameter("embs", [P, kE * T], BF16, isOutput=False)
    d_encT = nc.declare_dram_parameter("encT", [P, kE * S], BF16, isOutput=False)
    d_encC = nc.declare_dram_parameter("encC", [P, kS * E], BF16, isOutput=False)
    d_w1 = nc.declare_dram_parameter("w1", [P, kH * E], BF16, isOutput=False)
    d_w2 = nc.declare_dram_parameter("w2", [P, kE * H], BF16, isOutput=False)
    d_wh2e = nc.declare_dram_parameter("wh2e", [P, kH * E], BF16, isOutput=False)
    d_fcwb = nc.declare_dram_parameter("fcwb", [NCG, P, kE * GW], BF16, isOutput=False)
    d_cwb = nc.declare_dram_parameter(
        "cwb", [L, 2, KW, kH // KG, P, KG * H], BF16, isOutput=False
    )
    d_ones2 = nc.declare_dram_parameter("c_ones2", [P, P], BF16, isOutput=False)
    if with_bias:
        d_b1 = nc.declare_dram_parameter("b1", [1, E], BF16, isOutput=False)
        d_b2s2 = nc.declare_dram_parameter("b2s2", [H, 1], F32, isOutput=False)
        d_bh2e = nc.declare_dram_parameter("bh2e", [1, E], F32R, isOutput=False)
    d_out = nc.declare_dram_parameter("out", [T, V], BF16, isOutput=True)

    with tile.TileContext(nc) as tc, ExitStack() as ctx:
        pers = ctx.enter_context(tc.tile_pool(name="pers", bufs=1))
        pp = ctx.enter_context(tc.tile_pool(name="pp", bufs=8, space="PSUM"))

        # ---- persistent SBUF tensors -------------------------------------
        u = [
            pers.tile([P, T + KW - 1], F32R, tag=f"u{i}", name=f"u{i}")
            for i in range(kH)
        ]
        # DMA emission order = rough hardware arrival order; init inputs
        # (embT/we2h) land first so PE starts early; persistent attention
        # tensors are DMA'd from the deferred hook after layer-0's conv
        # weight stream.
        embs_big = pers.tile([P, kE * T], BF16, tag="embs", name="embs_big")
        embs_t = [embs_big[:, T * i : T * (i + 1)] for i in range(kE)]
        encT_big = pers.tile([P, kE * S], BF16, tag="encT", name="encT_big")
        encT_t = [encT_big[:, S * i : S * (i + 1)] for i in range(kE)]
        encC_big = pers.tile([P, kS * E], BF16, tag="encC", name="encC_big")
        encC_t = [encC_big[:, E * i : E * (i + 1)] for i in range(kS)]
        w1_big = pers.tile([P, kH * E], BF16, tag="w1", name="w1_big")
        w1_t = [w1_big[:, E * i : E * (i + 1)] for i in range(kH)]
        w2_big = pers.tile([P, kE * H], BF16, tag="w2", name="w2_big")
        w2_t = [w2_big[:, H * i : H * (i + 1)] for i in range(kE)]
        ones2 = pers.tile([P, P], BF16, tag="ones2", name="ones2")

        def _dma_persistent():
            for i in range(kH):
                nc.sync.dma_start(u[i], d_u0[:, UW * i : UW * (i + 1)])
            nc.sync.dma_start(w1_big, d_w1[:, :])
            nc.sync.dma_start(embs_big, d_embs[:, :])
            nc.sync.dma_start(encT_big, d_encT[:, :])
            nc.sync.dma_start(encC_big, d_encC[:, :])
            nc.sync.dma_start(w2_big, d_w2[:, :])
            nc.sync.dma_start(ones2, d_ones2[:, :])

        if with_bias:
            d_crow = nc.declare_dram_parameter("c_ones_row", [1, T], BF16, isOutput=False)
            ones_row = pers.tile([1, T], BF16, tag="ones_row", name="ones_row")
            nc.sync.dma_start(ones_row, d_crow[:, :])
            b1_t = pers.tile([1, E], BF16, tag="b1", name="b1_t")
            nc.sync.dma_start(b1_t, d_b1[:, :])
            b2s2_sb = []
            for m in range(kH):
                t = pers.tile([P, 1], F32, tag=f"b2s2_{m}", name=f"b2s2_{m}")
                nc.sync.dma_start(t, d_b2s2[P * m : P * (m + 1), :])
                b2s2_sb.append(t)
            bh2e_t = pers.tile([1, E], F32R, tag="bh2e", name="bh2e_t")
            nc.sync.dma_start(bh2e_t, d_bh2e[:, :])
            d_cbf = nc.declare_dram_parameter("cb_bf", [L, 2 * H], BF16, isOutput=False)
            cb_t = []
            for l in range(L):
                t = pers.tile([1, 2 * H], BF16, tag=f"cb{l}", name=f"cb_t{l}")
                nc.sync.dma_start(t, d_cbf[l : l + 1, :])
                cb_t.append(t)

        # ---- init: u0 = emb @ W_e2h computed host-side; ubf0 (bf16 copy)
        # streams first so layer-0 conv starts as soon as it lands ---------
        ubf_pers = ctx.enter_context(tc.tile_pool(name="ubf_p", bufs=kH))
        ubf = []
        for m in range(kH):
            t = ubf_pers.tile([P, T + KW - 1], BF16, tag="ubf", name=f"ubf0_{m}")
            nc.sync.dma_start(t, d_ubf0[:, UW * m : UW * (m + 1)])
            ubf.append(t)

        # ---- layer stack -------------------------------------------------
        n_stripes = KW * kH
        # k-major stripe order: stripe i needs ubf[i // KW], so the highest-k
        # ubf tiles (produced last by the previous layer's epilogue) are
        # needed latest — hides the att2->ubf chain at layer boundaries.
        stripes = [(k, kw) for k in range(kH) for kw in range(KW)]
        with (
            tc.tile_pool(name="wconv_p", bufs=9) as wconv_p,
            tc.tile_pool(name="sig_p", bufs=kH) as sig_p,
            tc.tile_pool(name="glu_p", bufs=kH) as glu_p,
            tc.tile_pool(name="comb_p", bufs=kE) as comb_p,
            tc.tile_pool(name="ex_p", bufs=kS) as ex_p,
            tc.tile_pool(name="att_p", bufs=kE) as att_p,
            tc.tile_pool(name="rec_p", bufs=2) as rec_p,
            tc.tile_pool(name="y_p", bufs=kH) as y_p,
        ):
            for l in range(L):
                u_bf = ubf
                # conv + GLU: g-half (gate) first, then a-half.  Weights come
                # in [P, KG*H] mega-tiles (8KB rows); emission kg-major to
                # match the k-major stripe consumption order.
                sig = []
                glu_s = []
                for half in (1, 0):  # 1 = gate channels [H:2H), 0 = a [0:H)
                    wtiles = {}
                    for kg in range(kH // KG):
                        for kw in range(KW):
                            wt = wconv_p.tile(
                                [P, KG * H], BF16, tag="wst",
                                name=f"wst{l}_{half}_{kw}_{kg}",
                            )
                            nc.sync.dma_start(wt, d_cwb[l, half, kw, kg, :, :])
                            wtiles[(kw, kg)] = wt
                    for m in range(kH):
                        cps = pp.tile([P, T], F32, tag="ps", name=f"cps{l}_{half}_{m}")
                        for i_mm, (k, kw) in enumerate(stripes):
                            wt = wtiles[(kw, k // KG)]
                            off = (k % KG) * H + P * m
                            nc.tensor.matmul(
                                cps,
                                wt[:, off : off + P],
                                u_bf[k][:, kw : kw + T],
                                start=(i_mm == 0),
                                stop=(i_mm == n_stripes - 1 and not with_bias),
                            )
                        if with_bias:
                            nc.tensor.matmul(
                                cps,
                                cb_t[l][
                                    :, half * H + P * m : half * H + P * (m + 1)
                                ],
                                ones_row,
                                start=False,
                                stop=True,
                            )
                        if half == 1:
                            sg = sig_p.tile([P, T], BF16, tag="sig", name=f"sig{l}_{m}")
                            nc.scalar.activation(sg, cps, AF.Sigmoid)
                            sig.append(sg)
                        else:
                            # glu_s = (a * S2) * sigmoid(g), stored bf16
                            g = glu_p.tile([P, T], BF16, tag="glu", name=f"glu{l}_{m}")
                            nc.vector.scalar_tensor_tensor(
                                g, cps, S2, sig[m], AluOpType.mult, AluOpType.mult
                            )
                            glu_s.append(g)

                if l == 0:
                    # persistent attention tensors arrive after layer-0's conv
                    # weight stream — they're first needed ~90us in
                    _dma_persistent()

                # attention: comb = (glu_s.T @ w1) * sqrt(2) + emb*SQ, (E,T)
                comb = []
                for m in range(kE):
                    ps = pp.tile([P, T], F32, tag="ps", name=f"ceps{l}_{m}")
                    for k in range(kH):
                        nc.tensor.matmul(
                            ps,
                            w1_t[k][:, P * m : P * (m + 1)],
                            glu_s[k],
                            start=(k == 0),
                            stop=(k == kH - 1 and not with_bias),
                        )
                    if with_bias:
                        nc.tensor.matmul(
                            ps,
                            b1_t[:, P * m : P * (m + 1)],
                            ones_row,
                            start=False,
                            stop=True,
                        )
                    c = comb_p.tile([P, T], BF16, tag="comb", name=f"comb{l}_{m}")
                    nc.vector.scalar_tensor_tensor(
                        c, ps, RT2, embs_t[m], AluOpType.mult, AluOpType.add
                    )
                    comb.append(c)

                # energy in (S, T) layout; exp elementwise (energies are
                # bounded ~|22| for this model, fp32-safe without max-sub)
                ex = []
                for m in range(kS):
                    ps = pp.tile([P, T], F32, tag="ps", name=f"enps{l}_{m}")
                    for k in range(kE):
                        nc.tensor.matmul(
                            ps,
                            encT_t[k][:, P * m : P * (m + 1)],
                            comb[k],
                            start=(k == 0),
                            stop=(k == kE - 1),
                        )
                    e = ex_p.tile([P, T], BF16, tag="ex", name=f"ex{l}_{m}")
                    nc.scalar.activation(e, ps, AF.Exp)
                    ex.append(e)

                # column sums over S via a 2.0-constant ones matmul: every
                # psum row = 2*sum, so the reciprocal runs full-partition and
                # directly yields rbc = 0.5/sums (the S2 factor folded in).
                sps = pp.tile([P, T], F32, tag="ps", name=f"sums{l}")
                for k in range(kS):
                    nc.tensor.matmul(
                        sps, ones2, ex[k], start=(k == 0), stop=(k == kS - 1)
                    )
                rbc = rec_p.tile([P, T], F32, tag="rbc", name=f"rbc{l}")
                with nc.allow_low_precision(reason="softmax recip feeds DVE mul"):
                    nc.vector.reciprocal(rbc, sps)

                # attended (E,T), unnormalized — normalization (x rbc) is
                # applied in the att2 epilogue
                att = []
                for m in range(kE):
                    ps = pp.tile([P, T], F32, tag="ps", name=f"atps{l}_{m}")
                    for k in range(kS):
                        nc.tensor.matmul(
                            ps,
                            encC_t[k][:, P * m : P * (m + 1)],
                            ex[k],
                            start=(k == 0),
                            stop=(k == kS - 1),
                        )
                    a = att_p.tile([P, T], BF16, tag="att", name=f"att{l}_{m}")
                    nc.scalar.copy(a, ps)
                    att.append(a)

                # att2 = w2.T @ att; per m-tile epilogue (engines split so no
                # single queue backs up):
                #   x1 = att2_psum * rbc           (DVE, psum operand)
                #   y  = glu_s + x1                (GPSIMD, sbuf only)
                #   u  = u*SQ + y                  (GPSIMD)
                #   ubf= bf16(u)                   (ACT even / DVE odd)
                next_ubf = []
                x1s = []
                for m in range(kH):
                    ps = pp.tile([P, T], F32, tag="ps", name=f"a2ps{l}_{m}")
                    for k in range(kE):
                        nc.tensor.matmul(
                            ps,
                            w2_t[k][:, P * m : P * (m + 1)],
                            att[k],
                            start=(k == 0),
                            stop=(k == kE - 1),
                        )
                    x1 = y_p.tile([P, T], F32, tag="x1", name=f"x1_{l}_{m}")
                    nc.vector.tensor_mul(x1, ps, rbc)
                    if with_bias:
                        nc.vector.tensor_scalar_add(x1, x1, b2s2_sb[m])
                    x1s.append(x1)
                for m in range(kH):
                    y = y_p.tile([P, T], BF16, tag="y", name=f"y{l}_{m}")
                    nc.gpsimd.tensor_add(y, glu_s[m], x1s[m])
                    nc.vector.scalar_tensor_tensor(
                        u[m][:, KW - 1 :],
                        u[m][:, KW - 1 :],
                        SQ,
                        y,
                        AluOpType.mult,
                        AluOpType.add,
                    )
                    nb = ubf_pers.tile(
                        [P, T + KW - 1], BF16, tag="ubf", name=f"ubf{l + 1}_{m}"
                    )
                    nc.scalar.copy(nb, u[m])
                    next_ubf.append(nb)
                ubf = next_ubf

        # ---- final: convout (E,T) then fc_out (T,V) ----------------------
        with (
            tc.tile_pool(name="wh2e_p", bufs=1) as wh2e_p,
            tc.tile_pool(name="co_p", bufs=1) as co_p,
            tc.tile_pool(name="fcw_p", bufs=4) as fcw_p,
            tc.tile_pool(name="ot_p", bufs=mT + 2) as ot_p,
        ):
            wh2e_big = wh2e_p.tile([P, kH * E], BF16, tag="wh2e", name="wh2e_big")
            nc.sync.dma_start(wh2e_big, d_wh2e[:, :])
            wh2e_t = [wh2e_big[:, E * i : E * (i + 1)] for i in range(kH)]
            co = []
            for m in range(kE):
                ps = pp.tile([P, T], F32, tag="ps", name=f"cops{m}")
                for k in range(kH):
                    nc.tensor.matmul(
                        ps,
                        wh2e_t[k][:, P * m : P * (m + 1)],
                        ubf[k][:, KW - 1 :],
                        start=(k == 0),
                        stop=(k == kH - 1 and not with_bias),
                    )
                if with_bias:
                    nc.tensor.matmul(
                        ps,
                        bh2e_t[:, P * m : P * (m + 1)],
                        ones_row,
                        start=False,
                        stop=True,
                    )
                t = co_p.tile([P, T], BF16, tag=f"co{m}", name=f"co{m}")
                nc.scalar.copy(t, ps)
                co.append(t)

            # fc weights stream in [P, kE*GW] mega-tiles (one DMA per chunk
            # group, 16KB rows), 2-deep explicit prefetch
            fts = {}

            def fetch(cg):
                ft = fcw_p.tile([P, kE * GW], BF16, tag="fcw", name=f"fcw{cg}")
                nc.sync.dma_start(ft, d_fcwb[cg, :, :])
                fts[cg] = ft

            fetch(0)
            if NCG > 1:
                fetch(1)
            for cg in range(NCG):
                if cg + 2 < NCG:
                    fetch(cg + 2)
                ft = fts.pop(cg)
                for m in range(mT):
                    ot = ot_p.tile([P, GW], BF16, tag="ot", name=f"ot{cg}_{m}")
                    for sub in range(GS):
                        ps = pp.tile([P, CH], F32, tag="ps", name=f"fcps{cg}_{m}_{sub}")
                        for k in range(kE):
                            nc.tensor.matmul(
                                ps,
                                co[k][:, P * m : P * (m + 1)],
                                ft[:, k * GW + CH * sub : k * GW + CH * (sub + 1)],
                                start=(k == 0),
                                stop=(k == kE - 1),
                            )
                        if cg == NCG - 1 and sub % 2 == 1:
                            nc.scalar.copy(ot[:, CH * sub : CH * (sub + 1)], ps)
                        else:
                            nc.vector.tensor_copy(ot[:, CH * sub : CH * (sub + 1)], ps)
                    nc.sync.dma_start(
                        d_out[P * m : P * (m + 1), GW * cg : GW * (cg + 1)], ot
                    )

    if legalize:
        _legalize_pe_waits(nc)
    return nc


def _host_prep(inp, T, L, KW):
    """Host-side input prep: embedding gather, transposes, weight relayouts."""
    import ml_dtypes

    f32 = np.float32
    bf16 = ml_dtypes.bfloat16
    trg = np.asarray(inp["trg"]).astype(np.int64)
    tok = np.asarray(inp["tok_emb"], dtype=f32)
    pos = np.asarray(inp["pos_emb"], dtype=f32)
    embedded = tok[trg] + pos[:T][None]  # (B,T,E)
    sq = f32(np.sqrt(np.float32(0.5)))
    def meg(x):
        # [B, K*128, W] -> [B, 128, K*W] mega-row layout (k-tiles side by side)
        Bb, KP, W = x.shape
        return np.ascontiguousarray(
            x.reshape(Bb, KP // 128, 128, W).transpose(0, 2, 1, 3)
        ).reshape(Bb, 128, (KP // 128) * W)

    we2h = np.asarray(inp["emb2hid_w"], dtype=f32)
    b_e2h = np.asarray(inp["emb2hid_b"], dtype=f32)
    u0 = (embedded @ we2h + b_e2h).transpose(0, 2, 1)  # (B, H, T) f32
    Bb, Hh = u0.shape[0], u0.shape[1]
    u0p = np.concatenate(
        [np.full((Bb, Hh, KW - 1), f32(1.0)), u0], axis=2
    )  # (B, H, T+KW-1), left pad = 1.0
    u0b = meg(np.ascontiguousarray(u0p))
    ubf0b = meg(np.ascontiguousarray(u0p).astype(bf16))
    embs = meg(np.ascontiguousarray((embedded * sq).transpose(0, 2, 1)).astype(bf16))
    encT = meg(np.ascontiguousarray(
        np.asarray(inp["encoder_conved"], dtype=f32).transpose(0, 2, 1)
    ).astype(bf16))
    encC = meg(np.ascontiguousarray(
        np.asarray(inp["encoder_combined"], dtype=f32)
    ).astype(bf16))

    cw = np.ascontiguousarray(
        np.asarray(inp["conv_w"], dtype=f32).transpose(0, 3, 2, 1)
    ).astype(bf16)  # (L, KW, H, 2H)
    KG = 4
    H = cw.shape[2]
    kH = H // 128
    # (L,KW,H,2H) -> mega-tile layout [l, half, kw, kg, p, kk*H + c]
    cwb = cw.reshape(L, KW, kH // KG, KG, 128, 2, H).transpose(0, 5, 1, 2, 4, 3, 6)
    cwb = np.ascontiguousarray(cwb).reshape(L, 2, KW, kH // KG, 128, KG * H)
    return u0b, ubf0b, embs, encT, encC, cwb


def _meg1(x):
    """[K*128, W] -> [128, K*W] mega-row layout."""
    KP, W = x.shape
    return np.ascontiguousarray(
        x.reshape(KP // 128, 128, W).transpose(1, 0, 2)
    ).reshape(128, (KP // 128) * W)


def kernel(**inputs):
    B, T, S = 8, 512, 512
    E, H, V = 512, 1024, 32000
    KW, L = 3, 6
    CH = 500
    GS = 4
    GW = GS * CH
    NCG = V // GW
    kE = E // P

    import ml_dtypes

    f32 = np.float32
    bf16 = ml_dtypes.bfloat16
    inp = {k: np.asarray(v) for k, v in inputs.items()}
    u0b, ubf0b, embs, encT, encC, cwb = _host_prep(inp, T, L, KW)

    dev_biases = ["emb2hid_b", "conv_b", "attn_hid2emb_b", "attn_emb2hid_b", "hid2emb_b"]
    with_bias = any(np.any(np.asarray(inp[k])) for k in dev_biases)

    nc = build_decoder_nc(
        T=T, S=S, E=E, H=H, V=V, L=L, KW=KW, CH=CH, with_bias=with_bias
    )

    fcw = np.asarray(inp["fc_out_w"], dtype=f32).astype(bf16)  # (E, V)
    fcwb = np.ascontiguousarray(
        fcw.reshape(kE, 128, NCG, GW).transpose(2, 1, 0, 3)
    ).reshape(NCG, 128, kE * GW)

    base = {
        "c_ones2": np.full((128, 128), 2.0, dtype=bf16),
        "w1": _meg1(np.asarray(inp["attn_hid2emb_w"], dtype=f32).astype(bf16)),
        "w2": _meg1(np.asarray(inp["attn_emb2hid_w"], dtype=f32).astype(bf16)),
        "wh2e": _meg1(np.asarray(inp["hid2emb_w"], dtype=f32).astype(bf16)),
        "fcwb": fcwb,
        "cwb": cwb,
    }
    if with_bias:
        base |= {
            "c_ones_row": np.ones((1, T), bf16),
            "b1": np.asarray(inp["attn_hid2emb_b"], dtype=f32).reshape(1, E).astype(bf16),
            "b2s2": (np.asarray(inp["attn_emb2hid_b"], dtype=f32) * f32(0.5)).reshape(H, 1),
            "bh2e": np.asarray(inp["hid2emb_b"], dtype=f32).reshape(1, E),
            "cb_bf": np.ascontiguousarray(np.asarray(inp["conv_b"], dtype=f32)).astype(bf16),
        }
    in_maps = [
        dict(base, u0=u0b[c], ubf0=ubf0b[c], embs=embs[c], encT=encT[c], encC=encC[c])
        for c in range(B)
    ]

    from concourse.bass_utils import run_bass_kernel_spmd

    import os

    trace = bool(os.environ.get("DECODER_TRACE"))
    res = run_bass_kernel_spmd(nc, in_maps, core_ids=list(range(B)), trace=trace)
    global _last_results
    _last_results = res
    out = np.stack([np.asarray(res.results[c]["out"]) for c in range(B)]).astype(f32)

    fcb = np.asarray(inp["fc_out_b"], dtype=f32)
    if np.any(fcb):
        out = out + fcb[None, None, :]
    return out


# revision 14
# speedup vs baseline: 1.0100x; 1.0100x over previous
"""Trainium2 Bass kernel for the ConvS2S-style decoder (nn_Decoder).

Strategy: pure data-parallel over batch — B=8 batch elements mapped 1:1 onto
8 NeuronCores, zero cross-core communication.  Host does the (tiny) embedding
gather + weight transposes/relayouts; each core runs the full per-batch
pipeline:

    u0 = emb @ W_e2h computed on HOST (numpy); u0 (f32) + its bf16 copy
    stream to the device, so layer-0 conv starts as soon as 1MB lands
    6 x [ conv(K=3, via 3 shifted matmuls) -> GLU
          -> attention (energy in (S,T) layout, softmax column sums via a
             2.0-constant ones matmul so the reciprocal runs full-partition,
             normalization folded into the att2 epilogue) -> residual updates ]
    convout = u.T @ W_h2e ; out = convout @ W_fc   (T,V layout, bf16 out)

All matmuls are bf16 x bf16.  Key perf detail: conv/fc weights stream from HBM in
mega-tiles with 8-16KB contiguous per-partition rows — small (2KB) DMA
descriptors at the conv's ~120GB/s stream rate slow PE SBUF reads by ~20%
(259ns vs 216ns per 512-col matmul).  The residual u stays f32; everything
else is bf16, halving HBM traffic vs f32.
"""

import numpy as np
from contextlib import ExitStack

import bass_rust
import concourse.bass as bass
import concourse.mybir as mybir
import concourse.tile as tile
from concourse.alu_op_type import AluOpType

F32 = mybir.dt.float32
F32R = mybir.dt.float32r
BF16 = mybir.dt.bfloat16
AF = mybir.ActivationFunctionType
P = 128

_last_results = None


def _legalize_pe_waits(nc):
    """Walrus packs a self-loading (fp32/fp32r) Matmult's sync waits into the
    LDWEIGHTS hw descriptor, which has a single wait slot.  Move the waits of
    any multi-wait PE compute instruction onto EventSemaphore instructions
    (one wait each) inserted just before it on the PE queue — semantically
    identical wait point, but each carrier is within the hw limit."""
    n = 0
    absorb_types = (
        "InstMatmult",
        "InstLdweights",
        "InstDMACopy",
        "InstActivation",
        "InstTensorTensor",
        "InstTensorScalarPtr",
        "InstTensorCopy",
        "InstReciprocal",
        "InstMemset",
        "InstTensorReduce",
        "InstDrain",
    )
    for fn in nc.m.functions:
        for blk in fn.blocks:
            out = []
            changed = False
            for inst in blk.instructions:
                si = inst.sync_info
                if (
                    si is not None
                    and type(inst).__name__ in absorb_types
                ):
                    waits = list(si.on_wait)
                    if len(waits) > 1:
                        for w in waits:
                            out.append(
                                mybir.InstEventSemaphore(
                                    name=f"I-pewait{n}",
                                    engine=inst.engine,
                                    sync_info=bass_rust.SyncInfo(
                                        on_wait=[w], on_update=[]
                                    ),
                                    ins=[],
                                    outs=[],
                                )
                            )
                            n += 1
                        inst.sync_info = bass_rust.SyncInfo(
                            on_wait=[], on_update=list(si.on_update)
                        )
                        changed = True
                out.append(inst)
            if changed:
                blk.instructions = out
    return n


def build_decoder_nc(T, S, E, H, V, L, KW, CH, with_bias, legalize=True):
    """Build the per-core Bass program.  All dims must be multiples of 128
    (except V which must be a multiple of CH, CH <= 512)."""
    kE, kH, kS, mT = E // P, H // P, S // P, T // P
    NCH = V // CH
    KG = 4  # k-tiles batched per conv weight mega-tile DMA
    GS = 4 if NCH % 4 == 0 else (2 if NCH % 2 == 0 else 1)
    GW = GS * CH
    NCG = NCH // GS
    SQ = float(np.sqrt(np.float32(0.5)))
    S2 = 0.5  # SQ**2 exactly
    RT2 = float(np.float32(np.sqrt(np.float32(2.0))))

    nc = bass.Bass()

    UW = T + KW - 1
    d_u0 = nc.declare_dram_parameter("u0", [P, kH * UW], F32R, isOutput=False)
    d_ubf0 = nc.declare_dram_parameter("ubf0", [P, kH * UW], BF16, isOutput=False)
    d_embs = nc.declare_dram_parameter("embs", [P, kE * T], BF16, isOutput=False)
    d_encT = nc.declare_dram_parameter("encT", [P, kE * S], BF16, isOutput=False)
    d_encC = nc.declare_dram_parameter("encC", [P, kS * E], BF16, isOutput=False)
    d_w1 = nc.declare_dram_parameter("w1", [P, kH * E], BF16, isOutput=False)
    d_w2 = nc.declare_dram_parameter("w2", [P, kE * H], BF16, isOutput=False)
    d_wh2e = nc.declare_dram_parameter("wh2e", [P, kH * E], BF16, isOutput=False)
    d_fcwb = nc.declare_dram_parameter("fcwb", [NCG, P, kE * GW], BF16, isOutput=False)
    d_cwb = nc.declare_dram_parameter(
        "cwb", [L, 2, KW, kH // KG, P, KG * H], BF16, isOutput=False
    )
    d_ones2 = nc.declare_dram_parameter("c_ones2", [P, P], BF16, isOutput=False)
    if with_bias:
        d_b1 = nc.declare_dram_parameter("b1", [1, E], BF16, isOutput=False)
        d_b2s2 = nc.declare_dram_parameter("b2s2", [H, 1], F32, isOutput=False)
        d_bh2e = nc.declare_dram_parameter("bh2e", [1, E], F32R, isOutput=False)
    d_out = nc.declare_dram_parameter("out", [T, V], BF16, isOutput=True)

    with tile.TileContext(nc) as tc, ExitStack() as ctx:
        pers = ctx.enter_context(tc.tile_pool(name="pers", bufs=1))
        pp = ctx.enter_context(tc.tile_pool(name="pp", bufs=8, space="PSUM"))

        # ---- persistent SBUF tensors -------------------------------------
        u = [
            pers.tile([P, T + KW - 1], F32R, tag=f"u{i}", name=f"u{i}")
            for i in range(kH)
        ]
        # DMA emission order = rough hardware arrival order; init inputs
        # (embT/we2h) land first so PE starts early; persistent attention
        # tensors are DMA'd from the deferred hook after layer-0's conv
        # weight stream.
        embs_big = pers.tile([P, kE * T], BF16, tag="embs", name="embs_big")
        embs_t = [embs_big[:, T * i : T * (i + 1)] for i in range(kE)]
        encT_big = pers.tile([P, kE * S], BF16, tag="encT", name="encT_big")
        encT_t = [encT_big[:, S * i : S * (i + 1)] for i in range(kE)]
        encC_big = pers.tile([P, kS * E], BF16, tag="encC", name="encC_big")
        encC_t = [encC_big[:, E * i : E * (i + 1)] for i in range(kS)]
        w1_big = pers.tile([P, kH * E], BF16, tag="w1", name="w1_big")
        w1_t = [w1_big[:, E * i : E * (i + 1)] for i in range(kH)]
        w2_big = pers.tile([P, kE * H], BF16, tag="w2", name="w2_big")
        w2_t = [w2_big[:, H * i : H * (i + 1)] for i in range(kE)]
        ones2 = pers.tile([P, P], BF16, tag="ones2", name="ones2")

        def _dma_persistent():
            for i in range(kH):
                nc.sync.dma_start(u[i], d_u0[:, UW * i : UW * (i + 1)])
            nc.sync.dma_start(w1_big, d_w1[:, :])
            nc.sync.dma_start(embs_big, d_embs[:, :])
            nc.sync.dma_start(encT_big, d_encT[:, :])
            nc.sync.dma_start(encC_big, d_encC[:, :])
            nc.sync.dma_start(w2_big, d_w2[:, :])
            nc.sync.dma_start(ones2, d_ones2[:, :])

        if with_bias:
            d_crow = nc.declare_dram_parameter("c_ones_row", [1, T], BF16, isOutput=False)
            ones_row = pers.tile([1, T], BF16, tag="ones_row", name="ones_row")
            nc.sync.dma_start(ones_row, d_crow[:, :])
            b1_t = pers.tile([1, E], BF16, tag="b1", name="b1_t")
            nc.sync.dma_start(b1_t, d_b1[:, :])
            b2s2_sb = []
            for m in range(kH):
                t = pers.tile([P, 1], F32, tag=f"b2s2_{m}", name=f"b2s2_{m}")
                nc.sync.dma_start(t, d_b2s2[P * m : P * (m + 1), :])
                b2s2_sb.append(t)
            bh2e_t = pers.tile([1, E], F32R, tag="bh2e", name="bh2e_t")
            nc.sync.dma_start(bh2e_t, d_bh2e[:, :])
            d_cbf = nc.declare_dram_parameter("cb_bf", [L, 2 * H], BF16, isOutput=False)
            cb_t = []
            for l in range(L):
                t = pers.tile([1, 2 * H], BF16, tag=f"cb{l}", name=f"cb_t{l}")
                nc.sync.dma_start(t, d_cbf[l : l + 1, :])
                cb_t.append(t)

        # ---- init: u0 = emb @ W_e2h computed host-side; ubf0 (bf16 copy)
        # streams first so layer-0 conv starts as soon as it lands ---------
        ubf_pers = ctx.enter_context(tc.tile_pool(name="ubf_p", bufs=kH))
        ubf = []
        for m in range(kH):
            t = ubf_pers.tile([P, T + KW - 1], BF16, tag="ubf", name=f"ubf0_{m}")
            nc.sync.dma_start(t, d_ubf0[:, UW * m : UW * (m + 1)])
            ubf.append(t)

        # ---- layer stack -------------------------------------------------
        n_stripes = KW * kH
        # k-major stripe order: stripe i needs ubf[i // KW], so the highest-k
        # ubf tiles (produced last by the previous layer's epilogue) are
        # needed latest — hides the att2->ubf chain at layer boundaries.
        stripes = [(k, kw) for k in range(kH) for kw in range(KW)]
        with (
            tc.tile_pool(name="wconv_p", bufs=9) as wconv_p,
            tc.tile_pool(name="sig_p", bufs=kH) as sig_p,
            tc.tile_pool(name="glu_p", bufs=kH) as glu_p,
            tc.tile_pool(name="comb_p", bufs=kE) as comb_p,
            tc.tile_pool(name="ex_p", bufs=kS) as ex_p,
            tc.tile_pool(name="att_p", bufs=kE) as att_p,
            tc.tile_pool(name="rec_p", bufs=2) as rec_p,
            tc.tile_pool(name="y_p", bufs=kH) as y_p,
        ):
            for l in range(L):
                u_bf = ubf
                # conv + GLU: g-half (gate) first, then a-half.  Weights come
                # in [P, KG*H] mega-tiles (8KB rows); emission kg-major to
                # match the k-major stripe consumption order.
                sig = []
                glu_s = []
                for half in (1, 0):  # 1 = gate channels [H:2H), 0 = a [0:H)
                    wtiles = {}
                    for kg in range(kH // KG):
                        for kw in range(KW):
                            wt = wconv_p.tile(
                                [P, KG * H], BF16, tag="wst",
                                name=f"wst{l}_{half}_{kw}_{kg}",
                            )
                            nc.sync.dma_start(wt, d_cwb[l, half, kw, kg, :, :])
                            wtiles[(kw, kg)] = wt
                    for m in range(kH):
                        cps = pp.tile([P, T], F32, tag="ps", name=f"cps{l}_{half}_{m}")
                        for i_mm, (k, kw) in enumerate(stripes):
                            wt = wtiles[(kw, k // KG)]
                            off = (k % KG) * H + P * m
                            nc.tensor.matmul(
                                cps,
                                wt[:, off : off + P],
                                u_bf[k][:, kw : kw + T],
                                start=(i_mm == 0),
                                stop=(i_mm == n_stripes - 1 and not with_bias),
                            )
                        if with_bias:
                            nc.tensor.matmul(
                                cps,
                                cb_t[l][
                                    :, half * H + P * m : half * H + P * (m + 1)
                                ],
                                ones_row,
                                start=False,
                                stop=True,
                            )
                        if half == 1:
                            sg = sig_p.tile([P, T], BF16, tag="sig", name=f"sig{l}_{m}")
                            nc.scalar.activation(sg, cps, AF.Sigmoid)
                            sig.append(sg)
                        else:
                            # glu_s = (a * S2) * sigmoid(g), stored bf16
                            g = glu_p.tile([P, T], BF16, tag="glu", name=f"glu{l}_{m}")
                            nc.vector.scalar_tensor_tensor(
                                g, cps, S2, sig[m], AluOpType.mult, AluOpType.mult
                            )
                            glu_s.append(g)

                if l == 0:
                    # persistent attention tensors arrive after layer-0's conv
                    # weight stream — they're first needed ~90us in
                    _dma_persistent()

                # attention: comb = (glu_s.T @ w1) * sqrt(2) + emb*SQ, (E,T)
                comb = []
                for m in range(kE):
                    ps = pp.tile([P, T], F32, tag="ps", name=f"ceps{l}_{m}")
                    for k in range(kH):
                        nc.tensor.matmul(
                            ps,
                            w1_t[k][:, P * m : P * (m + 1)],
                            glu_s[k],
                            start=(k == 0),
                            stop=(k == kH - 1 and not with_bias),
                        )
                    if with_bias:
                        nc.tensor.matmul(
                            ps,
                            b1_t[:, P * m : P * (m + 1)],
                            ones_row,
                            start=False,
                            stop=True,
                        )
                    c = comb_p.tile([P, T], BF16, tag="comb", name=f"comb{l}_{m}")
                    nc.vector.scalar_tensor_tensor(
                        c, ps, RT2, embs_t[m], AluOpType.mult, AluOpType.add
                    )
                    comb.append(c)

                # energy in (S, T) layout; exp elementwise (energies are
                # bounded ~|22| for this model, fp32-safe without max-sub)
                ex = []
                for m in range(kS):
                    ps = pp.tile([P, T], F32, tag="ps", name=f"enps{l}_{m}")
                    for k in range(kE):
                        nc.tensor.matmul(
                            ps,
                            encT_t[k][:, P * m : P * (m + 1)],
                            comb[k],
                            start=(k == 0),
                            stop=(k == kE - 1),
                        )
                    e = ex_p.tile([P, T], BF16, tag="ex", name=f"ex{l}_{m}")
                    nc.scalar.activation(e, ps, AF.Exp)
                    ex.append(e)

                # column sums over S via a 2.0-constant ones matmul: every
                # psum row = 2*sum, so the reciprocal runs full-partition and
                # directly yields rbc = 0.5/sums (the S2 factor folded in).
                sps = pp.tile([P, T], F32, tag="ps", name=f"sums{l}")
                for k in range(kS):
                    nc.tensor.matmul(
                        sps, ones2, ex[k], start=(k == 0), stop=(k == kS - 1)
                    )
                rbc = rec_p.tile([P, T], F32, tag="rbc", name=f"rbc{l}")
                with nc.allow_low_precision(reason="softmax recip feeds DVE mul"):
                    nc.vector.reciprocal(rbc, sps)

                # attended (E,T), normalized here (x rbc) so the att2 psum
                # comes out as the finished residual contribution
                att = []
                for m in range(kE):
                    ps = pp.tile([P, T], F32, tag="ps", name=f"atps{l}_{m}")
                    for k in range(kS):
                        nc.tensor.matmul(
                            ps,
                            encC_t[k][:, P * m : P * (m + 1)],
                            ex[k],
                            start=(k == 0),
                            stop=(k == kS - 1),
                        )
                    a = att_p.tile([P, T], BF16, tag="att", name=f"att{l}_{m}")
                    nc.vector.tensor_mul(a, ps, rbc)
                    att.append(a)
                # pre-fold the GLU term into the residual while DVE has slack:
                # u <- u*SQ + glu_s; the epilogue then just adds the att2 psum
                for m in range(kH):
                    nc.vector.scalar_tensor_tensor(
                        u[m][:, KW - 1 :],
                        u[m][:, KW - 1 :],
                        SQ,
                        glu_s[m],
                        AluOpType.mult,
                        AluOpType.add,
                    )
                    if with_bias:
                        nc.vector.tensor_scalar_add(
                            u[m][:, KW - 1 :], u[m][:, KW - 1 :], b2s2_sb[m]
                        )

                # att2 = w2.T @ att; per m-tile epilogue (engines split so no
                # single queue backs up):
                #   x1 = att2_psum * rbc           (DVE, psum operand)
                #   y  = glu_s + x1                (GPSIMD, sbuf only)
                #   u  = u*SQ + y                  (GPSIMD)
                #   ubf= bf16(u)                   (ACT even / DVE odd)
                # epilogue: att2 psum is already the normalized*S2 attended
                # contribution; u just accumulates it (8 DVE adds), then cast
                next_ubf = []
                for m in range(kH):
                    ps = pp.tile([P, T], F32, tag="ps", name=f"a2ps{l}_{m}")
                    for k in range(kE):
                        nc.tensor.matmul(
                            ps,
                            w2_t[k][:, P * m : P * (m + 1)],
                            att[k],
                            start=(k == 0),
                            stop=(k == kE - 1),
                        )
                    nc.vector.tensor_add(
                        u[m][:, KW - 1 :], u[m][:, KW - 1 :], ps
                    )
                    nb = ubf_pers.tile(
                        [P, T + KW - 1], BF16, tag="ubf", name=f"ubf{l + 1}_{m}"
                    )
                    nc.scalar.copy(nb, u[m])
                    next_ubf.append(nb)
                ubf = next_ubf

        # ---- final: convout (E,T) then fc_out (T,V) ----------------------
        with (
            tc.tile_pool(name="wh2e_p", bufs=1) as wh2e_p,
            tc.tile_pool(name="co_p", bufs=1) as co_p,
            tc.tile_pool(name="fcw_p", bufs=4) as fcw_p,
            tc.tile_pool(name="ot_p", bufs=mT + 2) as ot_p,
        ):
            wh2e_big = wh2e_p.tile([P, kH * E], BF16, tag="wh2e", name="wh2e_big")
            nc.sync.dma_start(wh2e_big, d_wh2e[:, :])
            wh2e_t = [wh2e_big[:, E * i : E * (i + 1)] for i in range(kH)]
            co = []
            for m in range(kE):
                ps = pp.tile([P, T], F32, tag="ps", name=f"cops{m}")
                for k in range(kH):
                    nc.tensor.matmul(
                        ps,
                        wh2e_t[k][:, P * m : P * (m + 1)],
                        ubf[k][:, KW - 1 :],
                        start=(k == 0),
                        stop=(k == kH - 1 and not with_bias),
                    )
                if with_bias:
                    nc.tensor.matmul(
                        ps,
                        bh2e_t[:, P * m : P * (m + 1)],
                        ones_row,
                        start=False,
                        stop=True,
                    )
                t = co_p.tile([P, T], BF16, tag=f"co{m}", name=f"co{m}")
                nc.scalar.copy(t, ps)
                co.append(t)

            # fc weights stream in [P, kE*GW] mega-tiles (one DMA per chunk
            # group, 16KB rows), 2-deep explicit prefetch
            fts = {}

            def fetch(cg):
                ft = fcw_p.tile([P, kE * GW], BF16, tag="fcw", name=f"fcw{cg}")
                nc.sync.dma_start(ft, d_fcwb[cg, :, :])
                fts[cg] = ft

            fetch(0)
            if NCG > 1:
                fetch(1)
            for cg in range(NCG):
                if cg + 2 < NCG:
                    fetch(cg + 2)
                ft = fts.pop(cg)
                for m in range(mT):
                    ot = ot_p.tile([P, GW], BF16, tag="ot", name=f"ot{cg}_{m}")
                    for sub in range(GS):
                        ps = pp.tile([P, CH], F32, tag="ps", name=f"fcps{cg}_{m}_{sub}")
                        for k in range(kE):
                            nc.tensor.matmul(
                                ps,
                                co[k][:, P * m : P * (m + 1)],
                                ft[:, k * GW + CH * sub : k * GW + CH * (sub + 1)],
                                start=(k == 0),
                                stop=(k == kE - 1),
                            )
                        if cg == NCG - 1 and sub % 2 == 1:
                            nc.scalar.copy(ot[:, CH * sub : CH * (sub + 1)], ps)
                        else:
                            nc.vector.tensor_copy(ot[:, CH * sub : CH * (sub + 1)], ps)
                    nc.sync.dma_start(
                        d_out[P * m : P * (m + 1), GW * cg : GW * (cg + 1)], ot
                    )

    if legalize:
        _legalize_pe_waits(nc)
    return nc


def _host_prep(inp, T, L, KW):
    """Host-side input prep: embedding gather, transposes, weight relayouts."""
    import ml_dtypes

    f32 = np.float32
    bf16 = ml_dtypes.bfloat16
    trg = np.asarray(inp["trg"]).astype(np.int64)
    tok = np.asarray(inp["tok_emb"], dtype=f32)
    pos = np.asarray(inp["pos_emb"], dtype=f32)
    embedded = tok[trg] + pos[:T][None]  # (B,T,E)
    sq = f32(np.sqrt(np.float32(0.5)))
    def meg(x):
        # [B, K*128, W] -> [B, 128, K*W] mega-row layout (k-tiles side by side)
        Bb, KP, W = x.shape
        return np.ascontiguousarray(
            x.reshape(Bb, KP // 128, 128, W).transpose(0, 2, 1, 3)
        ).reshape(Bb, 128, (KP // 128) * W)

    we2h = np.asarray(inp["emb2hid_w"], dtype=f32)
    b_e2h = np.asarray(inp["emb2hid_b"], dtype=f32)
    u0 = (embedded @ we2h + b_e2h).transpose(0, 2, 1)  # (B, H, T) f32
    Bb, Hh = u0.shape[0], u0.shape[1]
    u0p = np.concatenate(
        [np.full((Bb, Hh, KW - 1), f32(1.0)), u0], axis=2
    )  # (B, H, T+KW-1), left pad = 1.0
    u0b = meg(np.ascontiguousarray(u0p))
    ubf0b = meg(np.ascontiguousarray(u0p).astype(bf16))
    embs = meg(np.ascontiguousarray((embedded * sq).transpose(0, 2, 1)).astype(bf16))
    encT = meg(np.ascontiguousarray(
        np.asarray(inp["encoder_conved"], dtype=f32).transpose(0, 2, 1)
    ).astype(bf16))
    encC = meg(np.ascontiguousarray(
        np.asarray(inp["encoder_combined"], dtype=f32)
    ).astype(bf16))

    cw = np.ascontiguousarray(
        np.asarray(inp["conv_w"], dtype=f32).transpose(0, 3, 2, 1)
    ).astype(bf16)  # (L, KW, H, 2H)
    KG = 4
    H = cw.shape[2]
    kH = H // 128
    # (L,KW,H,2H) -> mega-tile layout [l, half, kw, kg, p, kk*H + c]
    cwb = cw.reshape(L, KW, kH // KG, KG, 128, 2, H).transpose(0, 5, 1, 2, 4, 3, 6)
    cwb = np.ascontiguousarray(cwb).reshape(L, 2, KW, kH // KG, 128, KG * H)
    return u0b, ubf0b, embs, encT, encC, cwb


def _meg1(x):
    """[K*128, W] -> [128, K*W] mega-row layout."""
    KP, W = x.shape
    return np.ascontiguousarray(
        x.reshape(KP // 128, 128, W).transpose(1, 0, 2)
    ).reshape(128, (KP // 128) * W)


def kernel(**inputs):
    B, T, S = 8, 512, 512
    E, H, V = 512, 1024, 32000
    KW, L = 3, 6
    CH = 500
    GS = 4
    GW = GS * CH
    NCG = V // GW
    kE = E // P

    import ml_dtypes

    f32 = np.float32
    bf16 = ml_dtypes.bfloat16
    inp = {k: np.asarray(v) for k, v in inputs.items()}
    u0b, ubf0b, embs, encT, encC, cwb = _host_prep(inp, T, L, KW)

    dev_biases = ["emb2hid_b", "conv_b", "attn_hid2emb_b", "attn_emb2hid_b", "hid2emb_b"]
    with_bias = any(np.any(np.asarray(inp[k])) for k in dev_biases)

    nc = build_decoder_nc(
        T=T, S=S, E=E, H=H, V=V, L=L, KW=KW, CH=CH, with_bias=with_bias
    )

    fcw = np.asarray(inp["fc_out_w"], dtype=f32).astype(bf16)  # (E, V)
    fcwb = np.ascontiguousarray(
        fcw.reshape(kE, 128, NCG, GW).transpose(2, 1, 0, 3)
    ).reshape(NCG, 128, kE * GW)

    base = {
        "c_ones2": np.full((128, 128), 2.0, dtype=bf16),
        "w1": _meg1(np.asarray(inp["attn_hid2emb_w"], dtype=f32).astype(bf16)),
        "w2": _meg1(np.asarray(inp["attn_emb2hid_w"], dtype=f32).astype(bf16)),
        "wh2e": _meg1(np.asarray(inp["hid2emb_w"], dtype=f32).astype(bf16)),
        "fcwb": fcwb,
        "cwb": cwb,
    }
    if with_bias:
        base |= {
            "c_ones_row": np.ones((1, T), bf16),
            "b1": np.asarray(inp["attn_hid2emb_b"], dtype=f32).reshape(1, E).astype(bf16),
            "b2s2": (np.asarray(inp["attn_emb2hid_b"], dtype=f32) * f32(0.5)).reshape(H, 1),
            "bh2e": np.asarray(inp["hid2emb_b"], dtype=f32).reshape(1, E),
            "cb_bf": np.ascontiguousarray(np.asarray(inp["conv_b"], dtype=f32)).astype(bf16),
        }
    in_maps = [
        dict(base, u0=u0b[c], ubf0=ubf0b[c], embs=embs[c], encT=encT[c], encC=encC[c])
        for c in range(B)
    ]

    from concourse.bass_utils import run_bass_kernel_spmd

    import os

    trace = bool(os.environ.get("DECODER_TRACE"))
    res = run_bass_kernel_spmd(nc, in_maps, core_ids=list(range(B)), trace=trace)
    global _last_results
    _last_results = res
    out = np.stack([np.asarray(res.results[c]["out"]) for c in range(B)]).astype(f32)

    fcb = np.asarray(inp["fc_out_b"], dtype=f32)
    if np.any(fcb):
        out = out + fcb[None, None, :]
    return out


# revision 16
# speedup vs baseline: 1.0138x; 1.0037x over previous
"""Trainium2 Bass kernel for the ConvS2S-style decoder (nn_Decoder).

Strategy: pure data-parallel over batch — B=8 batch elements mapped 1:1 onto
8 NeuronCores, zero cross-core communication.  Host does the (tiny) embedding
gather + weight transposes/relayouts; each core runs the full per-batch
pipeline:

    u0 = emb @ W_e2h computed on HOST (numpy); u0 (f32) + its bf16 copy
    stream to the device, so layer-0 conv starts as soon as 1MB lands
    6 x [ conv(K=3, via 3 shifted matmuls) -> GLU
          -> attention (energy in (S,T) layout, softmax column sums via a
             2.0-constant ones matmul so the reciprocal runs full-partition,
             normalization folded into the att2 epilogue) -> residual updates ]
    convout = u.T @ W_h2e ; out = convout @ W_fc   (T,V layout, bf16 out)

All matmuls are bf16 x bf16.  Key perf detail: conv/fc weights stream from HBM in
mega-tiles with 8-16KB contiguous per-partition rows — small (2KB) DMA
descriptors at the conv's ~120GB/s stream rate slow PE SBUF reads by ~20%
(259ns vs 216ns per 512-col matmul).  The residual u stays f32; everything
else is bf16, halving HBM traffic vs f32.
"""

import numpy as np
from contextlib import ExitStack

import bass_rust
import concourse.bass as bass
import concourse.mybir as mybir
import concourse.tile as tile
from concourse.alu_op_type import AluOpType

F32 = mybir.dt.float32
F32R = mybir.dt.float32r
BF16 = mybir.dt.bfloat16
AF = mybir.ActivationFunctionType
P = 128

_last_results = None


def _legalize_pe_waits(nc):
    """Walrus packs a self-loading (fp32/fp32r) Matmult's sync waits into the
    LDWEIGHTS hw descriptor, which has a single wait slot.  Move the waits of
    any multi-wait PE compute instruction onto EventSemaphore instructions
    (one wait each) inserted just before it on the PE queue — semantically
    identical wait point, but each carrier is within the hw limit."""
    n = 0
    absorb_types = (
        "InstMatmult",
        "InstLdweights",
        "InstDMACopy",
        "InstActivation",
        "InstTensorTensor",
        "InstTensorScalarPtr",
        "InstTensorCopy",
        "InstReciprocal",
        "InstMemset",
        "InstTensorReduce",
        "InstDrain",
    )
    for fn in nc.m.functions:
        for blk in fn.blocks:
            out = []
            changed = False
            for inst in blk.instructions:
                si = inst.sync_info
                if (
                    si is not None
                    and type(inst).__name__ in absorb_types
                ):
                    waits = list(si.on_wait)
                    if len(waits) > 1:
                        for w in waits:
                            out.append(
                                mybir.InstEventSemaphore(
                                    name=f"I-pewait{n}",
                                    engine=inst.engine,
                                    sync_info=bass_rust.SyncInfo(
                                        on_wait=[w], on_update=[]
                                    ),
                                    ins=[],
                                    outs=[],
                                )
                            )
                            n += 1
                        inst.sync_info = bass_rust.SyncInfo(
                            on_wait=[], on_update=list(si.on_update)
                        )
                        changed = True
                out.append(inst)
            if changed:
                blk.instructions = out
    return n


def build_decoder_nc(T, S, E, H, V, L, KW, CH, with_bias, legalize=True):
    """Build the per-core Bass program.  All dims must be multiples of 128
    (except V which must be a multiple of CH, CH <= 512)."""
    kE, kH, kS, mT = E // P, H // P, S // P, T // P
    NCH = V // CH
    KG = 4  # k-tiles batched per conv weight mega-tile DMA
    GS = 4 if NCH % 4 == 0 else (2 if NCH % 2 == 0 else 1)
    GW = GS * CH
    NCG = NCH // GS
    SQ = float(np.sqrt(np.float32(0.5)))
    S2 = 0.5  # SQ**2 exactly
    RT2 = float(np.float32(np.sqrt(np.float32(2.0))))

    nc = bass.Bass()

    UW = T + KW - 1
    d_u0 = nc.declare_dram_parameter("u0", [P, kH * UW], F32R, isOutput=False)
    d_ubf0 = nc.declare_dram_parameter("ubf0", [P, kH * UW], BF16, isOutput=False)
    d_embs = nc.declare_dram_parameter("embs", [P, kE * T], BF16, isOutput=False)
    d_encT = nc.declare_dram_parameter("encT", [P, kE * S], BF16, isOutput=False)
    d_encC = nc.declare_dram_parameter("encC", [P, kS * E], BF16, isOutput=False)
    d_w1 = nc.declare_dram_parameter("w1", [P, kH * E], BF16, isOutput=False)
    d_w2 = nc.declare_dram_parameter("w2", [P, kE * H], BF16, isOutput=False)
    d_wh2e = nc.declare_dram_parameter("wh2e", [P, kH * E], BF16, isOutput=False)
    d_fcwb = nc.declare_dram_parameter("fcwb", [NCG, P, kE * GW], BF16, isOutput=False)
    d_cwb = nc.declare_dram_parameter(
        "cwb", [L, 2, KW, kH // KG, P, KG * H], BF16, isOutput=False
    )
    d_ones2 = nc.declare_dram_parameter("c_ones2", [P, P], BF16, isOutput=False)
    if with_bias:
        d_b1 = nc.declare_dram_parameter("b1", [1, E], BF16, isOutput=False)
        d_b2s2 = nc.declare_dram_parameter("b2s2", [H, 1], F32, isOutput=False)
        d_bh2e = nc.declare_dram_parameter("bh2e", [1, E], F32R, isOutput=False)
    d_out = nc.declare_dram_parameter("out", [T, V], BF16, isOutput=True)

    with tile.TileContext(nc) as tc, ExitStack() as ctx:
        pers = ctx.enter_context(tc.tile_pool(name="pers", bufs=1))
        pp = ctx.enter_context(tc.tile_pool(name="pp", bufs=8, space="PSUM"))

        # ---- persistent SBUF tensors -------------------------------------
        u = [
            pers.tile([P, T + KW - 1], F32R, tag=f"u{i}", name=f"u{i}")
            for i in range(kH)
        ]
        # DMA emission order = rough hardware arrival order; init inputs
        # (embT/we2h) land first so PE starts early; persistent attention
        # tensors are DMA'd from the deferred hook after layer-0's conv
        # weight stream.
        embs_big = pers.tile([P, kE * T], BF16, tag="embs", name="embs_big")
        embs_t = [embs_big[:, T * i : T * (i + 1)] for i in range(kE)]
        encT_big = pers.tile([P, kE * S], BF16, tag="encT", name="encT_big")
        encT_t = [encT_big[:, S * i : S * (i + 1)] for i in range(kE)]
        encC_big = pers.tile([P, kS * E], BF16, tag="encC", name="encC_big")
        encC_t = [encC_big[:, E * i : E * (i + 1)] for i in range(kS)]
        w1_big = pers.tile([P, kH * E], BF16, tag="w1", name="w1_big")
        w1_t = [w1_big[:, E * i : E * (i + 1)] for i in range(kH)]
        w2_big = pers.tile([P, kE * H], BF16, tag="w2", name="w2_big")
        w2_t = [w2_big[:, H * i : H * (i + 1)] for i in range(kE)]
        ones2 = pers.tile([P, P], BF16, tag="ones2", name="ones2")

        def _dma_persistent():
            for i in range(kH):
                nc.sync.dma_start(u[i], d_u0[:, UW * i : UW * (i + 1)])
            nc.sync.dma_start(w1_big, d_w1[:, :])
            nc.sync.dma_start(embs_big, d_embs[:, :])
            nc.sync.dma_start(encT_big, d_encT[:, :])
            nc.sync.dma_start(encC_big, d_encC[:, :])
            nc.sync.dma_start(w2_big, d_w2[:, :])
            nc.sync.dma_start(ones2, d_ones2[:, :])

        if with_bias:
            d_crow = nc.declare_dram_parameter("c_ones_row", [1, T], BF16, isOutput=False)
            ones_row = pers.tile([1, T], BF16, tag="ones_row", name="ones_row")
            nc.sync.dma_start(ones_row, d_crow[:, :])
            b1_t = pers.tile([1, E], BF16, tag="b1", name="b1_t")
            nc.sync.dma_start(b1_t, d_b1[:, :])
            b2s2_sb = []
            for m in range(kH):
                t = pers.tile([P, 1], F32, tag=f"b2s2_{m}", name=f"b2s2_{m}")
                nc.sync.dma_start(t, d_b2s2[P * m : P * (m + 1), :])
                b2s2_sb.append(t)
            bh2e_t = pers.tile([1, E], F32R, tag="bh2e", name="bh2e_t")
            nc.sync.dma_start(bh2e_t, d_bh2e[:, :])
            d_cbf = nc.declare_dram_parameter("cb_bf", [L, 2 * H], BF16, isOutput=False)
            cb_t = []
            for l in range(L):
                t = pers.tile([1, 2 * H], BF16, tag=f"cb{l}", name=f"cb_t{l}")
                nc.sync.dma_start(t, d_cbf[l : l + 1, :])
                cb_t.append(t)

        # ---- init: u0 = emb @ W_e2h computed host-side; ubf0 (bf16 copy)
        # streams first so layer-0 conv starts as soon as it lands ---------
        ubf_pers = ctx.enter_context(tc.tile_pool(name="ubf_p", bufs=kH))
        ubf = []
        for m in range(kH):
            t = ubf_pers.tile([P, T + KW - 1], BF16, tag="ubf", name=f"ubf0_{m}")
            nc.sync.dma_start(t, d_ubf0[:, UW * m : UW * (m + 1)])
            ubf.append(t)

        # ---- layer stack -------------------------------------------------
        n_stripes = KW * kH
        # k-major stripe order: stripe i needs ubf[i // KW], so the highest-k
        # ubf tiles (produced last by the previous layer's epilogue) are
        # needed latest — hides the att2->ubf chain at layer boundaries.
        stripes = [(k, kw) for k in range(kH) for kw in range(KW)]
        with (
            tc.tile_pool(name="wconv_p", bufs=9) as wconv_p,
            tc.tile_pool(name="sig_p", bufs=kH) as sig_p,
            tc.tile_pool(name="glu_p", bufs=kH) as glu_p,
            tc.tile_pool(name="comb_p", bufs=kE) as comb_p,
            tc.tile_pool(name="ex_p", bufs=kS) as ex_p,
            tc.tile_pool(name="att_p", bufs=kE) as att_p,
            tc.tile_pool(name="rec_p", bufs=2) as rec_p,
            tc.tile_pool(name="y_p", bufs=kH) as y_p,
        ):
            for l in range(L):
                u_bf = ubf
                # conv + GLU: g-half (gate) first, then a-half.  Weights come
                # in [P, KG*H] mega-tiles (8KB rows); emission kg-major to
                # match the k-major stripe consumption order.
                sig = []
                glu_s = []
                for half in (1, 0):  # 1 = gate channels [H:2H), 0 = a [0:H)
                    wtiles = {}
                    for kg in range(kH // KG):
                        for kw in range(KW):
                            wt = wconv_p.tile(
                                [P, KG * H], BF16, tag="wst",
                                name=f"wst{l}_{half}_{kw}_{kg}",
                            )
                            nc.sync.dma_start(wt, d_cwb[l, half, kw, kg, :, :])
                            wtiles[(kw, kg)] = wt
                    for m in range(kH):
                        cps = pp.tile([P, T], F32, tag="ps", name=f"cps{l}_{half}_{m}")
                        for i_mm, (k, kw) in enumerate(stripes):
                            wt = wtiles[(kw, k // KG)]
                            off = (k % KG) * H + P * m
                            nc.tensor.matmul(
                                cps,
                                wt[:, off : off + P],
                                u_bf[k][:, kw : kw + T],
                                start=(i_mm == 0),
                                stop=(i_mm == n_stripes - 1 and not with_bias),
                            )
                        if with_bias:
                            nc.tensor.matmul(
                                cps,
                                cb_t[l][
                                    :, half * H + P * m : half * H + P * (m + 1)
                                ],
                                ones_row,
                                start=False,
                                stop=True,
                            )
                        if half == 1:
                            sg = sig_p.tile([P, T], BF16, tag="sig", name=f"sig{l}_{m}")
                            nc.scalar.activation(sg, cps, AF.Sigmoid)
                            sig.append(sg)
                        else:
                            # glu_s = (a * S2) * sigmoid(g), stored bf16
                            g = glu_p.tile([P, T], BF16, tag="glu", name=f"glu{l}_{m}")
                            nc.vector.scalar_tensor_tensor(
                                g, cps, S2, sig[m], AluOpType.mult, AluOpType.mult
                            )
                            glu_s.append(g)

                if l == 0:
                    # persistent attention tensors arrive after layer-0's conv
                    # weight stream — they're first needed ~90us in
                    _dma_persistent()

                # attention: comb = (glu_s.T @ w1) * sqrt(2) + emb*SQ, (E,T)
                comb = []
                for m in range(kE):
                    ps = pp.tile([P, T], F32, tag="ps", name=f"ceps{l}_{m}")
                    for k in range(kH):
                        nc.tensor.matmul(
                            ps,
                            w1_t[k][:, P * m : P * (m + 1)],
                            glu_s[k],
                            start=(k == 0),
                            stop=(k == kH - 1 and not with_bias),
                        )
                    if with_bias:
                        nc.tensor.matmul(
                            ps,
                            b1_t[:, P * m : P * (m + 1)],
                            ones_row,
                            start=False,
                            stop=True,
                        )
                    c = comb_p.tile([P, T], BF16, tag="comb", name=f"comb{l}_{m}")
                    nc.vector.scalar_tensor_tensor(
                        c, ps, RT2, embs_t[m], AluOpType.mult, AluOpType.add
                    )
                    comb.append(c)

                # energy in (S, T) layout; exp elementwise (energies are
                # bounded ~|22| for this model, fp32-safe without max-sub)
                ex = []
                for m in range(kS):
                    ps = pp.tile([P, T], F32, tag="ps", name=f"enps{l}_{m}")
                    for k in range(kE):
                        nc.tensor.matmul(
                            ps,
                            encT_t[k][:, P * m : P * (m + 1)],
                            comb[k],
                            start=(k == 0),
                            stop=(k == kE - 1),
                        )
                    e = ex_p.tile([P, T], BF16, tag="ex", name=f"ex{l}_{m}")
                    nc.scalar.activation(e, ps, AF.Exp)
                    ex.append(e)

                # column sums over S via a 2.0-constant ones matmul: every
                # psum row = 2*sum, so the reciprocal runs full-partition and
                # directly yields rbc = 0.5/sums (the S2 factor folded in).
                sps = pp.tile([P, T], F32, tag="ps", name=f"sums{l}")
                for k in range(kS):
                    nc.tensor.matmul(
                        sps, ones2, ex[k], start=(k == 0), stop=(k == kS - 1)
                    )
                rbc = rec_p.tile([P, T], F32, tag="rbc", name=f"rbc{l}")
                with nc.allow_low_precision(reason="softmax recip feeds DVE mul"):
                    nc.vector.reciprocal(rbc, sps)

                # attended (E,T), normalized here (x rbc) so the att2 psum
                # comes out as the finished residual contribution
                att = []
                for m in range(kE):
                    ps = pp.tile([P, T], F32, tag="ps", name=f"atps{l}_{m}")
                    for k in range(kS):
                        nc.tensor.matmul(
                            ps,
                            encC_t[k][:, P * m : P * (m + 1)],
                            ex[k],
                            start=(k == 0),
                            stop=(k == kS - 1),
                        )
                    a = att_p.tile([P, T], BF16, tag="att", name=f"att{l}_{m}")
                    nc.vector.tensor_mul(a, ps, rbc)
                    att.append(a)
                # pre-fold the GLU term into the residual while DVE has slack:
                # u <- u*SQ + glu_s; the epilogue then just adds the att2 psum
                for m in range(kH):
                    nc.vector.scalar_tensor_tensor(
                        u[m][:, KW - 1 :],
                        u[m][:, KW - 1 :],
                        SQ,
                        glu_s[m],
                        AluOpType.mult,
                        AluOpType.add,
                    )
                    if with_bias:
                        nc.vector.tensor_scalar_add(
                            u[m][:, KW - 1 :], u[m][:, KW - 1 :], b2s2_sb[m]
                        )

                # att2 = w2.T @ att; per m-tile epilogue (engines split so no
                # single queue backs up):
                #   x1 = att2_psum * rbc           (DVE, psum operand)
                #   y  = glu_s + x1                (GPSIMD, sbuf only)
                #   u  = u*SQ + y                  (GPSIMD)
                #   ubf= bf16(u)                   (ACT even / DVE odd)
                # epilogue: att2 psum is already the normalized*S2 attended
                # contribution; u just accumulates it (8 DVE adds), then cast
                next_ubf = []
                for m in range(kH):
                    ps = pp.tile([P, T], F32, tag="ps", name=f"a2ps{l}_{m}")
                    for k in range(kE):
                        nc.tensor.matmul(
                            ps,
                            w2_t[k][:, P * m : P * (m + 1)],
                            att[k],
                            start=(k == 0),
                            stop=(k == kE - 1),
                        )
                    nc.vector.tensor_add(
                        u[m][:, KW - 1 :], u[m][:, KW - 1 :], ps
                    )
                    nb = ubf_pers.tile(
                        [P, T + KW - 1], BF16, tag="ubf", name=f"ubf{l + 1}_{m}"
                    )
                    nc.scalar.copy(nb, u[m])
                    next_ubf.append(nb)
                ubf = next_ubf

        # ---- final: convout (E,T) then fc_out (T,V) ----------------------
        with (
            tc.tile_pool(name="wh2e_p", bufs=1) as wh2e_p,
            tc.tile_pool(name="co_p", bufs=1) as co_p,
            tc.tile_pool(name="fcw_p", bufs=4) as fcw_p,
            tc.tile_pool(name="ot_p", bufs=mT + 2) as ot_p,
        ):
            wh2e_big = wh2e_p.tile([P, kH * E], BF16, tag="wh2e", name="wh2e_big")
            nc.sync.dma_start(wh2e_big, d_wh2e[:, :])
            wh2e_t = [wh2e_big[:, E * i : E * (i + 1)] for i in range(kH)]
            co = []
            for m in range(kE):
                ps = pp.tile([P, T], F32, tag="ps", name=f"cops{m}")
                for k in range(kH):
                    nc.tensor.matmul(
                        ps,
                        wh2e_t[k][:, P * m : P * (m + 1)],
                        ubf[k][:, KW - 1 :],
                        start=(k == 0),
                        stop=(k == kH - 1 and not with_bias),
                    )
                if with_bias:
                    nc.tensor.matmul(
                        ps,
                        bh2e_t[:, P * m : P * (m + 1)],
                        ones_row,
                        start=False,
                        stop=True,
                    )
                t = co_p.tile([P, T], BF16, tag=f"co{m}", name=f"co{m}")
                nc.scalar.copy(t, ps)
                co.append(t)

            # fc weights stream in [P, kE*GW] mega-tiles (one DMA per chunk
            # group, 16KB rows), 2-deep explicit prefetch
            fts = {}

            def fetch(cg):
                ft = fcw_p.tile([P, kE * GW], BF16, tag="fcw", name=f"fcw{cg}")
                nc.sync.dma_start(ft, d_fcwb[cg, :, :])
                fts[cg] = ft

            fetch(0)
            if NCG > 1:
                fetch(1)
            for cg in range(NCG):
                if cg + 2 < NCG:
                    fetch(cg + 2)
                ft = fts.pop(cg)
                for m in range(mT):
                    ot = ot_p.tile([P, GW], BF16, tag="ot", name=f"ot{cg}_{m}")
                    for sub in range(GS):
                        ps = pp.tile([P, CH], F32, tag="ps", name=f"fcps{cg}_{m}_{sub}")
                        for k in range(kE):
                            nc.tensor.matmul(
                                ps,
                                co[k][:, P * m : P * (m + 1)],
                                ft[:, k * GW + CH * sub : k * GW + CH * (sub + 1)],
                                start=(k == 0),
                                stop=(k == kE - 1),
                            )
                        if cg == NCG - 1 and sub % 2 == 1:
                            nc.scalar.copy(ot[:, CH * sub : CH * (sub + 1)], ps)
                        else:
                            nc.vector.tensor_copy(ot[:, CH * sub : CH * (sub + 1)], ps)
                    nc.sync.dma_start(
                        d_out[P * m : P * (m + 1), GW * cg : GW * (cg + 1)], ot
                    )

    if legalize:
        _legalize_pe_waits(nc)
    return nc


def _host_prep(inp, T, L, KW):
    """Host-side input prep: embedding gather, transposes, weight relayouts."""
    import ml_dtypes

    f32 = np.float32
    bf16 = ml_dtypes.bfloat16
    trg = np.asarray(inp["trg"]).astype(np.int64)
    tok = np.asarray(inp["tok_emb"], dtype=f32)
    pos = np.asarray(inp["pos_emb"], dtype=f32)
    embedded = tok[trg] + pos[:T][None]  # (B,T,E)
    sq = f32(np.sqrt(np.float32(0.5)))
    def meg(x):
        # [B, K*128, W] -> [B, 128, K*W] mega-row layout (k-tiles side by side)
        Bb, KP, W = x.shape
        return np.ascontiguousarray(
            x.reshape(Bb, KP // 128, 128, W).transpose(0, 2, 1, 3)
        ).reshape(Bb, 128, (KP // 128) * W)

    we2h = np.asarray(inp["emb2hid_w"], dtype=f32)
    b_e2h = np.asarray(inp["emb2hid_b"], dtype=f32)
    u0 = (embedded @ we2h + b_e2h).transpose(0, 2, 1)  # (B, H, T) f32
    Bb, Hh = u0.shape[0], u0.shape[1]
    u0p = np.concatenate(
        [np.full((Bb, Hh, KW - 1), f32(1.0)), u0], axis=2
    )  # (B, H, T+KW-1), left pad = 1.0
    u0b = meg(np.ascontiguousarray(u0p))
    ubf0b = meg(np.ascontiguousarray(u0p).astype(bf16))
    embs = meg(np.ascontiguousarray((embedded * sq).transpose(0, 2, 1)).astype(bf16))
    encT = meg(np.ascontiguousarray(
        np.asarray(inp["encoder_conved"], dtype=f32).transpose(0, 2, 1)
    ).astype(bf16))
    encC = meg(np.ascontiguousarray(
        np.asarray(inp["encoder_combined"], dtype=f32)
    ).astype(bf16))

    cw = np.ascontiguousarray(
        np.asarray(inp["conv_w"], dtype=f32).transpose(0, 3, 2, 1)
    ).astype(bf16)  # (L, KW, H, 2H)
    KG = 4
    H = cw.shape[2]
    kH = H // 128
    # (L,KW,H,2H) -> mega-tile layout [l, half, kw, kg, p, kk*H + c]
    cwb = cw.reshape(L, KW, kH // KG, KG, 128, 2, H).transpose(0, 5, 1, 2, 4, 3, 6)
    cwb = np.ascontiguousarray(cwb).reshape(L, 2, KW, kH // KG, 128, KG * H)
    return u0b, ubf0b, embs, encT, encC, cwb


def _meg1(x):
    """[K*128, W] -> [128, K*W] mega-row layout."""
    KP, W = x.shape
    return np.ascontiguousarray(
        x.reshape(KP // 128, 128, W).transpose(1, 0, 2)
    ).reshape(128, (KP // 128) * W)


def kernel(**inputs):
    B, T, S = 8, 512, 512
    E, H, V = 512, 1024, 32000
    KW, L = 3, 6
    CH = 500
    GS = 4
    GW = GS * CH
    NCG = V // GW
    kE = E // P

    import ml_dtypes

    f32 = np.float32
    bf16 = ml_dtypes.bfloat16
    inp = {k: np.asarray(v) for k, v in inputs.items()}
    u0b, ubf0b, embs, encT, encC, cwb = _host_prep(inp, T, L, KW)

    dev_biases = ["emb2hid_b", "conv_b", "attn_hid2emb_b", "attn_emb2hid_b", "hid2emb_b"]
    with_bias = any(np.any(np.asarray(inp[k])) for k in dev_biases)

    nc = build_decoder_nc(
        T=T, S=S, E=E, H=H, V=V, L=L, KW=KW, CH=CH, with_bias=with_bias
    )

    fcw = np.asarray(inp["fc_out_w"], dtype=f32).astype(bf16)  # (E, V)
    fcwb = np.ascontiguousarray(
        fcw.reshape(kE, 128, NCG, GW).transpose(2, 1, 0, 3)
    ).reshape(NCG, 128, kE * GW)

    base = {
        "c_ones2": np.full((128, 128), 2.0, dtype=bf16),
        "w1": _meg1(np.asarray(inp["attn_hid2emb_w"], dtype=f32).astype(bf16)),
        "w2": _meg1(np.asarray(inp["attn_emb2hid_w"], dtype=f32).astype(bf16)),
        "wh2e": _meg1(np.asarray(inp["hid2emb_w"], dtype=f32).astype(bf16)),
        "fcwb": fcwb,
        "cwb": cwb,
    }
    if with_bias:
        base |= {
            "c_ones_row": np.ones((1, T), bf16),
            "b1": np.asarray(inp["attn_hid2emb_b"], dtype=f32).reshape(1, E).astype(bf16),
            "b2s2": (np.asarray(inp["attn_emb2hid_b"], dtype=f32) * f32(0.5)).reshape(H, 1),
            "bh2e": np.asarray(inp["hid2emb_b"], dtype=f32).reshape(1, E),
            "cb_bf": np.ascontiguousarray(np.asarray(inp["conv_b"], dtype=f32)).astype(bf16),
        }
    in_maps = [
        dict(base, u0=u0b[c], ubf0=ubf0b[c], embs=embs[c], encT=encT[c], encC=encC[c])
        for c in range(B)
    ]

    from concourse.bass_utils import run_bass_kernel_spmd

    import os

    trace = bool(os.environ.get("DECODER_TRACE"))
    res = run_bass_kernel_spmd(nc, in_maps, core_ids=list(range(B)), trace=trace)
    global _last_results
    _last_results = res
    out = np.stack([np.asarray(res.results[c]["out"]) for c in range(B)]).astype(f32)

    fcb = np.asarray(inp["fc_out_b"], dtype=f32)
    if np.any(fcb):
        out = out + fcb[None, None, :]
    return out
